# revision 6
# baseline (speedup 1.0000x reference)
"""Bi-path windowed attention kernel for Trainium2 (8 NeuronCores).

Problem: x (4, 512, 128, 128) f32. Reference (per batch): raw-reshape to
tokens (128,128,512); global path = 2x2-window MHA (8 heads, hd=64) +
out-proj; local path = AvgPool2(x) -> raw-reshape tokens (64,64,512) ->
2x2-window MHA -> raw-reshape -> reflect-pad smoothing along y and x ->
bilinear x2 upsample; out = (global + local) transposed to (B, C, H, W).

Sharding: 8 shards = batch (4) x channel-half (2). Channel half
[256h, 256h+256) of x == token rows [64h, 64h+64), and the local path
for those rows only touches those channels, so shards are independent.
Each core: xs = x[b, 256h:256h+256] -> out[b, :, 64h:64h+64, :].

Layout: activations token-major, tokens grouped window-major
(partition = window (Ip, J); free = (di, dj, c)). Matmuls in float32r
(TF32-like; full PE rate at N=512) with stationary operands from PE
transposes. Attention is DVE/ACT/GPSIMD elementwise math over the free
dim (q.k scores in bf16, rest f32). Local-path pooling / smoothing /
upsample run channel-major, exchanging with token-major stages through
DRAM scratch.
"""
import sys
sys.path.insert(0, '/opt/trn_rl_repo')
import numpy as np

_RUN_CACHE = {}

B, C, H, W = 4, 512, 128, 128
NH, HD = 8, 64


def _mk_tile_context_fixed():
    import concourse.mybir as mybir
    import concourse.tile as tile
    from concourse.vector_clock import ScopedClock, VectorClock

    class TileContextFixed(tile.TileContext):
        """Works around a walrus codegen limit in this toolchain: max ONE
        sync-wait per instruction. Extra waits are peeled onto single-wait
        NoOps on the same engine; the kernel-tail drain gets per-proc
        single-wait NOPs instead of one multi-wait drain."""
        _ctr = 0

        def _lower_ordered_insts(self, ordered):
            cls = type(self)
            for bb_name, insts in ordered.items():
                new_list = []
                for inst in insts:
                    try:
                        si = inst.sync_info
                    except Exception:
                        si = None
                    if si is not None and len(si.on_wait) > 1:
                        waits = list(si.on_wait)
                        extra, keep = waits[:-1], waits[-1:]
                        for w in extra:
                            nop = mybir.InstNoOp(
                                name=f"I-waitsplit-{cls._ctr}", ins=[], outs=[])
                            cls._ctr += 1
                            nop.engine = inst.engine
                            nop.sync_info = mybir.SyncInfo(
                                on_wait=[w], on_update=[])
                            self.nc.register_instruction(nop, overwrite=True)
                            new_list.append(nop)
                        inst.sync_info = mybir.SyncInfo(
                            on_wait=keep, on_update=list(si.on_update))
                    new_list.append(inst)
                ordered[bb_name] = new_list
            super()._lower_ordered_insts(ordered)

        def _drain_and_barrier(self, tick_clock, wait_clock):
            gc = tick_clock.global_clock
            scoped = gc if hasattr(gc, 'items') else ScopedClock({None: gc})
            for scope, vc in scoped.items():
                n = len(vc)
                for proc in range(n):
                    t = vc[proc]
                    if t <= 0:
                        continue
                    vec = [0] * n
                    vec[proc] = t
                    nop = self.nc.sync.nop()
                    wait_clock.add_sem_waits(
                        nop.ins, ScopedClock({scope: VectorClock(vec)}))
            self.nc.sync.drain()
            self.nc.all_engine_barrier()
            popped = self.nc._tile_sem_poison_stack.pop()
            assert popped is self._sem_poison
            self.nc.clear_and_free_semaphores(
                list(self.sems.allocated().values()))
            self.nc.all_engine_barrier()

    return TileContextFixed


def _dap(handle, off, dims):
    """Raw DRAM access pattern: flat element offset + [step, count] dims."""
    import concourse.bass as bass
    base = handle[:]
    return bass.AP(tensor=base.tensor, offset=base.offset + off,
                   ap=[list(d) for d in dims])


def _sap(tile_, off, dims):
    """SBUF tile sub-AP: keep partition dim, replace free dims."""
    import concourse.bass as bass
    base = tile_[:]
    return bass.AP(tensor=base.tensor, offset=base.offset + off,
                   ap=[list(base.ap[0])] + [list(d) for d in dims])


def _build_nc():
    import concourse.bass as bass
    import concourse.mybir as mybir
    from concourse.masks import make_identity
    TileContextFixed = _mk_tile_context_fixed()
    f32 = mybir.dt.float32
    f32r = mybir.dt.float32r
    bf16 = mybir.dt.bfloat16
    Copy = mybir.ActivationFunctionType.Copy
    Exp = mybir.ActivationFunctionType.Exp
    ADD = mybir.AluOpType.add
    MUL = mybir.AluOpType.mult
    AX = mybir.AxisListType.X
    THIRD = 1.0 / 3.0

    nc = bass.Bass()
    xs = nc.declare_dram_parameter("xs", [256, H, W], f32, isOutput=False)
    wqkv = nc.declare_dram_parameter("Wqkv", [C, 3 * C], f32, isOutput=False)
    bqkv = nc.declare_dram_parameter("bqkv", [3 * C], f32, isOutput=False)
    wproj = nc.declare_dram_parameter("Wproj", [C, C], f32, isOutput=False)
    bproj = nc.declare_dram_parameter("bproj", [C], f32, isOutput=False)
    out = nc.declare_dram_parameter("out", [C, 64, W], f32, isOutput=True)

    pooled = nc.dram_tensor("pooled", [256 * 64 * 64], f32)
    lout = nc.dram_tensor("lout", [2048 * 512], f32)
    lup = nc.dram_tensor("lup", [256 * H * W], f32)

    with TileContextFixed(nc) as tc:
        with (
            tc.tile_pool(name="consts", bufs=1) as consts,
            tc.tile_pool(name="work", bufs=2) as work,
            tc.tile_pool(name="psum", bufs=2, space="PSUM") as psum,
        ):
            # ---- constants ----
            ident = consts.tile([128, 128], f32)
            make_identity(nc, ident[:])
            wqkv_r = consts.tile([128, 4, 1536], f32r)
            wproj_r = consts.tile([128, 4, 512], f32r)
            for kc in range(4):
                wst = work.tile([128, 1536], f32, tag="xwm")
                nc.sync.dma_start(out=wst, in_=_dap(
                    wqkv, kc * 128 * 1536, [[1536, 128], [1, 1536]]))
                nc.vector.tensor_copy(wqkv_r[:, kc, :], wst[:])
                wst2 = work.tile([128, 512], f32, tag="lupg")
                nc.sync.dma_start(out=wst2, in_=_dap(
                    wproj, kc * 128 * 512, [[512, 128], [1, 512]]))
                nc.vector.tensor_copy(wproj_r[:, kc, :], wst2[:])
            bqkv_b = consts.tile([128, 1536], f32)
            nc.sync.dma_start(out=bqkv_b, in_=_dap(bqkv, 0, [[0, 128], [1, 1536]]))
            bproj_b = consts.tile([128, 512], f32)
            nc.sync.dma_start(out=bproj_b, in_=_dap(bproj, 0, [[0, 128], [1, 512]]))

            # ---- stage B: avg-pool 2x2 channel-major -> pooled scratch ----
            # strips of 16 input rows -> 8 pooled rows each
            for cc in range(2):
                for yt in range(8):
                    pin = work.tile([128, 2048], f32, tag="xwm")
                    nc.sync.dma_start(out=pin, in_=_dap(
                        xs, cc * 128 * 16384 + yt * 16 * 128,
                        [[16384, 128], [1, 2048]]))
                    t1 = work.tile([128, 512], f32, tag="lupg")
                    nc.vector.tensor_add(
                        t1[:], _sap(pin, 0, [[256, 8], [2, 64]]),
                        _sap(pin, 1, [[256, 8], [2, 64]]))
                    t2 = work.tile([128, 512], f32, tag="g1")
                    nc.vector.tensor_add(
                        t2[:], _sap(pin, 128, [[256, 8], [2, 64]]),
                        _sap(pin, 129, [[256, 8], [2, 64]]))
                    t3 = work.tile([128, 512], f32, tag="t3")
                    nc.gpsimd.tensor_add(t3[:], t1[:], t2[:])
                    t4 = work.tile([128, 512], f32, tag="outt")
                    nc.scalar.activation(t4[:], t3[:], Copy, scale=0.25)
                    nc.sync.dma_start(
                        out=_dap(pooled, cc * 128 * 4096 + yt * 512,
                                 [[4096, 128], [1, 512]]),
                        in_=t4)

            # ---- shared qkv + attention emitter (window-major tile) ----
            def qkv_attn_tile(x_wm):
                """x_wm: [128 win, 2048] = (di, dj, c). Returns O [128,2048] f32."""
                xt = work.tile([128, 4, 4, 128], f32r, tag="xt")
                for i in range(4):
                    for kc in range(4):
                        psT = psum.tile([128, 128], f32, tag="psT")
                        nc.tensor.transpose(
                            psT[:], _sap(x_wm, i * 512 + kc * 128, [[1, 128]]),
                            ident[:])
                        nc.scalar.copy(out=xt[:, i, kc, :], in_=psT[:])
                qk = work.tile([128, 4, 1024], bf16, tag="qk")
                vt = work.tile([128, 4, 512], f32, tag="vt")
                for i in range(4):
                    for nb in range(3):
                        psQ = psum.tile([128, 512], f32, tag="psQ")
                        for kc in range(4):
                            nc.tensor.matmul(
                                psQ[:], xt[:, i, kc, :],
                                wqkv_r[:, kc, nb * 512:(nb + 1) * 512],
                                start=(kc == 0), stop=(kc == 3))
                        dst = (qk[:, i, 0:512] if nb == 0 else
                               qk[:, i, 512:1024] if nb == 1 else vt[:, i, :])
                        nc.vector.tensor_add(
                            dst, psQ[:], bqkv_b[:, nb * 512:(nb + 1) * 512])
                # scores S[w,i,n,j] = sum_d q[i,n,d]*k[j,n,d]  (bf16 in, f32 out)
                S = work.tile([128, 128], f32, tag="S")
                tmpb = work.tile([128, 2048], bf16, tag="tmpb")
                for j in range(4):
                    k_j = _sap(qk, j * 1024 + 512, [[0, 4], [64, 8], [1, 64]])
                    q_all = _sap(qk, 0, [[1024, 4], [64, 8], [1, 64]])
                    nc.vector.tensor_mul(tmpb[:], q_all, k_j)
                    nc.vector.tensor_reduce(
                        out=_sap(S, j, [[32, 4], [4, 8]]),
                        in_=_sap(tmpb, 0, [[512, 4], [64, 8], [1, 64]]),
                        axis=AX, op=ADD)
                E = work.tile([128, 128], f32, tag="E")
                nc.scalar.activation(E[:], S[:], Exp, scale=float(HD) ** -0.5)
                D = work.tile([128, 32], f32, tag="D")
                nc.vector.tensor_reduce(
                    out=D[:], in_=_sap(E, 0, [[4, 32], [1, 4]]), axis=AX, op=ADD)
                R = work.tile([128, 32], f32, tag="R")
                nc.vector.reciprocal(R[:], D[:])
                P = work.tile([128, 128], f32, tag="P")
                nc.vector.tensor_mul(P[:], E[:], _sap(R, 0, [[1, 32], [0, 4]]))
                # O[w,i,n,d] = sum_j P[w,i,n,j] * v[w,j,n,d]
                O = work.tile([128, 2048], f32, tag="O")
                tmpO = work.tile([128, 2048], f32, tag="tmpO")
                for j in range(4):
                    p_j = _sap(P, j, [[32, 4], [4, 8], [0, 64]])
                    v_j = _sap(vt, j * 512, [[0, 4], [64, 8], [1, 64]])
                    if j == 0:
                        nc.vector.tensor_mul(O[:], p_j, v_j)
                    else:
                        nc.vector.tensor_mul(tmpO[:], p_j, v_j)
                        nc.gpsimd.tensor_add(O[:], O[:], tmpO[:])
                return O

            # ---- stage C: local attention (4 ltiles) -> lout scratch ----
            for lt in range(4):
                x_l = work.tile([128, 2, 1024], f32, tag="xwm")
                for di in range(2):
                    nc.sync.dma_start(out=x_l[:, di, :], in_=_dap(
                        pooled, lt * 262144 + di * 32768,
                        [[65536, 4], [1024, 32], [1, 1024]]))
                O_l = qkv_attn_tile(x_l)
                for di in range(2):
                    nc.sync.dma_start(
                        out=_dap(lout, lt * 262144 + di * 32768,
                                 [[65536, 4], [1024, 32], [1, 1024]]),
                        in_=_sap(O_l, di * 1024, [[1, 1024]]))

            # ---- stage D: smoothing + bilinear x2 upsample -> lup scratch ----
            # All scale factors folded algebraically: with raw sums
            #   a1[y] = l[y-1]+l[y] (reflect y=0), a2[y] = l[y]+l[y+1] (reflect
            #   x=63), sraw = a1+a2 (= 2*(lx+ly))
            #   u'[2y] = sraw[y] + sraw[y-1]/3, u'[2y+1] = sraw[y] + sraw[y+1]/3
            #   L'[2x] = u'[x] + u'[x-1]/3,  L'[2x+1] = u'[x] + u'[x+1]/3
            #   lup = 0.28125 * L'   (= 0.5 * 0.375 * 0.75 * ... collapsed)
            # borders use clamped taps; the stt form works there unchanged.
            for cc in range(2):
                for st in range(4):          # strips of 16 pooled rows
                    y0 = st * 16
                    r0, r1 = max(y0 - 2, 0), min(y0 + 17, 64)   # Lp rows
                    s0, s1 = max(y0 - 1, 0), min(y0 + 17, 64)   # sraw rows
                    nlr = r1 - r0
                    nsr = s1 - s0
                    Lp = work.tile([128, nlr * 64], f32, tag="xwm")
                    nc.sync.dma_start(out=Lp, in_=_dap(
                        lout, cc * 128 * 4096 + r0 * 64,
                        [[4096, 128], [1, nlr * 64]]))

                    def lrow(y):  # strip-local Lp row offset
                        return (y - r0) * 64

                    def srow(y):  # strip-local sraw row offset
                        return (y - s0) * 64

                    a1 = work.tile([128, nsr * 64], f32, tag="tmpO")
                    ym = max(s0, 1)  # main region rows [ym, s1)
                    nc.vector.tensor_add(
                        _sap(a1, srow(ym), [[1, (s1 - ym) * 64]]),
                        _sap(Lp, lrow(ym - 1), [[1, (s1 - ym) * 64]]),
                        _sap(Lp, lrow(ym), [[1, (s1 - ym) * 64]]))
                    if s0 == 0:  # reflect top: a1[0] = l[0] + l[1]
                        nc.vector.tensor_add(
                            _sap(a1, 0, [[1, 64]]),
                            _sap(Lp, 0, [[1, 64]]),
                            _sap(Lp, 64, [[1, 64]]))
                    a2 = work.tile([128, nsr * 64], f32, tag="O")
                    nc.gpsimd.tensor_add(
                        _sap(a2, 0, [[64, nsr], [1, 63]]),
                        _sap(Lp, lrow(s0), [[64, nsr], [1, 63]]),
                        _sap(Lp, lrow(s0) + 1, [[64, nsr], [1, 63]]))
                    nc.gpsimd.tensor_add(
                        _sap(a2, 63, [[64, nsr]]),
                        _sap(Lp, lrow(s0) + 63, [[64, nsr]]),
                        _sap(Lp, lrow(s0) + 62, [[64, nsr]]))
                    sraw = work.tile([128, nsr * 64], f32, tag="xwm")
                    nc.vector.tensor_add(sraw[:], a1[:], a2[:])
                    # y-upsample (u' rows Y-2*y0, 32 rows x 64 cols)
                    u = work.tile([128, 2048], f32, tag="vt")
                    ye = max(y0, 1)  # even rows needing y-1
                    nc.vector.scalar_tensor_tensor(
                        out=_sap(u, (ye - y0) * 128, [[128, y0 + 16 - ye], [1, 64]]),
                        in0=_sap(sraw, srow(ye - 1), [[64, y0 + 16 - ye], [1, 64]]),
                        scalar=THIRD,
                        in1=_sap(sraw, srow(ye), [[64, y0 + 16 - ye], [1, 64]]),
                        op0=MUL, op1=ADD)
                    if y0 == 0:  # Y=0: taps both row 0
                        nc.vector.scalar_tensor_tensor(
                            out=_sap(u, 0, [[1, 64]]),
                            in0=_sap(sraw, 0, [[1, 64]]), scalar=THIRD,
                            in1=_sap(sraw, 0, [[1, 64]]), op0=MUL, op1=ADD)
                    yo1 = min(y0 + 16, 63)  # odd rows needing y+1: y in [y0, yo1)
                    nc.vector.scalar_tensor_tensor(
                        out=_sap(u, 64, [[128, yo1 - y0], [1, 64]]),
                        in0=_sap(sraw, srow(y0 + 1), [[64, yo1 - y0], [1, 64]]),
                        scalar=THIRD,
                        in1=_sap(sraw, srow(y0), [[64, yo1 - y0], [1, 64]]),
                        op0=MUL, op1=ADD)
                    if y0 + 16 == 64:  # Y=127: taps both row 63
                        nc.vector.scalar_tensor_tensor(
                            out=_sap(u, 31 * 64, [[1, 64]]),
                            in0=_sap(sraw, srow(63), [[1, 64]]), scalar=THIRD,
                            in1=_sap(sraw, srow(63), [[1, 64]]), op0=MUL, op1=ADD)
                    # x-upsample per 16-row half + final scale + store
                    for hf in range(2):
                        Lh = work.tile([128, 2048], f32, tag="O")
                        ub = hf * 16 * 64  # u offset of this half's rows
                        nc.vector.scalar_tensor_tensor(
                            out=_sap(Lh, 2, [[128, 16], [2, 63]]),
                            in0=_sap(u, ub, [[64, 16], [1, 63]]), scalar=THIRD,
                            in1=_sap(u, ub + 1, [[64, 16], [1, 63]]),
                            op0=MUL, op1=ADD)
                        nc.vector.scalar_tensor_tensor(
                            out=_sap(Lh, 0, [[128, 16]]),
                            in0=_sap(u, ub, [[64, 16]]), scalar=THIRD,
                            in1=_sap(u, ub, [[64, 16]]), op0=MUL, op1=ADD)
                        nc.vector.scalar_tensor_tensor(
                            out=_sap(Lh, 1, [[128, 16], [2, 63]]),
                            in0=_sap(u, ub + 1, [[64, 16], [1, 63]]), scalar=THIRD,
                            in1=_sap(u, ub, [[64, 16], [1, 63]]),
                            op0=MUL, op1=ADD)
                        nc.vector.scalar_tensor_tensor(
                            out=_sap(Lh, 127, [[128, 16]]),
                            in0=_sap(u, ub + 63, [[64, 16]]), scalar=THIRD,
                            in1=_sap(u, ub + 63, [[64, 16]]), op0=MUL, op1=ADD)
                        Lsc = work.tile([128, 2048], f32, tag="tmpO")
                        nc.scalar.activation(Lsc[:], Lh[:], Copy, scale=0.28125)
                        nc.sync.dma_start(
                            out=_dap(lup,
                                     cc * 128 * 16384 + (2 * y0 + 16 * hf) * 128,
                                     [[16384, 128], [1, 2048]]),
                            in_=Lsc)

            # ---- stage A: global path (16 wtiles) ----
            for ti in range(16):
                x_wm = work.tile([128, 2, 1024], f32, tag="xwm")
                for di in range(2):
                    nc.sync.dma_start(out=x_wm[:, di, :], in_=_dap(
                        xs, ti * 4 * 65536 + di * 65536,
                        [[131072, 2], [1024, 64], [1, 1024]]))
                O = qkv_attn_tile(x_wm)
                sums = work.tile([128, 4, 512], f32, tag="sums")
                for i in range(4):
                    at = work.tile([128, 4, 128], f32r, tag="at")
                    for kc in range(4):
                        psT2 = psum.tile([128, 128], f32, tag="psT")
                        nc.tensor.transpose(
                            psT2[:], _sap(O, i * 512 + kc * 128, [[1, 128]]),
                            ident[:])
                        nc.scalar.copy(out=at[:, kc, :], in_=psT2[:])
                    psP = psum.tile([128, 512], f32, tag="psQ")
                    for kc in range(4):
                        nc.tensor.matmul(psP[:], at[:, kc, :], wproj_r[:, kc, :],
                                         start=(kc == 0), stop=(kc == 3))
                    di, dj = i >> 1, i & 1
                    lupg = work.tile([128, 512], f32, tag="lupg")
                    nc.sync.dma_start(out=lupg, in_=_dap(
                        lup, ti * 262144 + di * 65536 + dj * 512,
                        [[131072, 2], [1024, 64], [1, 512]]))
                    g1 = work.tile([128, 512], f32, tag="g1")
                    nc.vector.tensor_add(g1[:], psP[:], bproj_b[:])
                    nc.gpsimd.tensor_add(sums[:, i, :], g1[:], lupg[:])
                # final transpose to (C, h, w) + DMA out
                for ch in range(4):
                    outt = work.tile([128, 512], f32, tag="outt")
                    for i in range(4):
                        di, dj = i >> 1, i & 1
                        psF = psum.tile([128, 128], f32, tag="psT")
                        nc.tensor.transpose(
                            psF[:], _sap(sums, i * 512 + ch * 128, [[1, 128]]),
                            ident[:])
                        nc.scalar.copy(
                            out=_sap(outt, di * 128 + dj, [[256, 2], [2, 64]]),
                            in_=_sap(psF, 0, [[64, 2], [1, 64]]))
                    nc.sync.dma_start(
                        out=_dap(out, ch * 128 * 8192 + ti * 512,
                                 [[8192, 128], [128, 4], [1, 128]]),
                        in_=outt)
    return nc


def _get_nc():
    if 'nc' not in _RUN_CACHE:
        _RUN_CACHE['nc'] = _build_nc()
    return _RUN_CACHE['nc']


def kernel(**inputs):
    from concourse.bass_utils import run_bass_kernel_spmd
    x = np.ascontiguousarray(np.asarray(inputs['x'], dtype=np.float32))
    Wqkv = np.ascontiguousarray(np.asarray(inputs['Wqkv'], dtype=np.float32))
    bqkv = np.ascontiguousarray(np.asarray(inputs['bqkv'], dtype=np.float32))
    Wproj = np.ascontiguousarray(np.asarray(inputs['Wproj'], dtype=np.float32))
    bproj = np.ascontiguousarray(np.asarray(inputs['bproj'], dtype=np.float32))

    nc = _get_nc()
    in_maps = []
    shards = []
    for b in range(B):
        for half in range(2):
            shards.append((b, half))
            in_maps.append({
                "xs": np.ascontiguousarray(x[b, 256 * half:256 * (half + 1)]),
                "Wqkv": Wqkv, "bqkv": bqkv, "Wproj": Wproj, "bproj": bproj,
            })
    r = run_bass_kernel_spmd(nc, in_maps, core_ids=list(range(8)))
    _RUN_CACHE['last_result'] = r
    full = np.empty((B, C, H, W), dtype=np.float32)
    for (b, half), res in zip(shards, r.results):
        full[b, :, 64 * half:64 * (half + 1), :] = res["out"]
    return full


# revision 7
# speedup vs baseline: 1.1277x; 1.1277x over previous
"""Bi-path windowed attention kernel for Trainium2 (8 NeuronCores).

Problem: x (4, 512, 128, 128) f32. Reference (per batch): raw-reshape to
tokens (128,128,512); global path = 2x2-window MHA (8 heads, hd=64) +
out-proj; local path = AvgPool2(x) -> raw-reshape tokens (64,64,512) ->
2x2-window MHA -> raw-reshape -> reflect-pad smoothing along y and x ->
bilinear x2 upsample; out = (global + local) transposed to (B, C, H, W).

Sharding: 8 shards = batch (4) x channel-half (2). Channel half
[256h, 256h+256) of x == token rows [64h, 64h+64), and the local path
for those rows only touches those channels, so shards are independent.
Each core: xs = x[b, 256h:256h+256] -> out[b, :, 64h:64h+64, :].

Layout: activations token-major, tokens grouped window-major
(partition = window (Ip, J); free = (di, dj, c)). Matmuls in float32r
(TF32-like; full PE rate at N=512) with stationary operands from PE
transposes. Attention is DVE/ACT/GPSIMD elementwise math over the free
dim (q.k scores in bf16, rest f32). Local-path pooling / smoothing /
upsample run channel-major, exchanging with token-major stages through
DRAM scratch.
"""
import sys
sys.path.insert(0, '/opt/trn_rl_repo')
import numpy as np

_RUN_CACHE = {}

B, C, H, W = 4, 512, 128, 128
NH, HD = 8, 64


def _mk_tile_context_fixed():
    import concourse.mybir as mybir
    import concourse.tile as tile
    from concourse.vector_clock import ScopedClock, VectorClock

    class TileContextFixed(tile.TileContext):
        """Works around a walrus codegen limit in this toolchain: max ONE
        sync-wait per instruction. Extra waits are peeled onto single-wait
        NoOps on the same engine; the kernel-tail drain gets per-proc
        single-wait NOPs instead of one multi-wait drain."""
        _ctr = 0

        def _lower_ordered_insts(self, ordered):
            cls = type(self)
            for bb_name, insts in ordered.items():
                new_list = []
                for inst in insts:
                    try:
                        si = inst.sync_info
                    except Exception:
                        si = None
                    if si is not None and len(si.on_wait) > 1:
                        waits = list(si.on_wait)
                        extra, keep = waits[:-1], waits[-1:]
                        for w in extra:
                            nop = mybir.InstNoOp(
                                name=f"I-waitsplit-{cls._ctr}", ins=[], outs=[])
                            cls._ctr += 1
                            nop.engine = inst.engine
                            nop.sync_info = mybir.SyncInfo(
                                on_wait=[w], on_update=[])
                            self.nc.register_instruction(nop, overwrite=True)
                            new_list.append(nop)
                        inst.sync_info = mybir.SyncInfo(
                            on_wait=keep, on_update=list(si.on_update))
                    new_list.append(inst)
                ordered[bb_name] = new_list
            super()._lower_ordered_insts(ordered)

        def _drain_and_barrier(self, tick_clock, wait_clock):
            gc = tick_clock.global_clock
            scoped = gc if hasattr(gc, 'items') else ScopedClock({None: gc})
            for scope, vc in scoped.items():
                n = len(vc)
                for proc in range(n):
                    t = vc[proc]
                    if t <= 0:
                        continue
                    vec = [0] * n
                    vec[proc] = t
                    nop = self.nc.sync.nop()
                    wait_clock.add_sem_waits(
                        nop.ins, ScopedClock({scope: VectorClock(vec)}))
            self.nc.sync.drain()
            self.nc.all_engine_barrier()
            popped = self.nc._tile_sem_poison_stack.pop()
            assert popped is self._sem_poison
            self.nc.clear_and_free_semaphores(
                list(self.sems.allocated().values()))
            self.nc.all_engine_barrier()

    return TileContextFixed


def _dap(handle, off, dims):
    """Raw DRAM access pattern: flat element offset + [step, count] dims."""
    import concourse.bass as bass
    base = handle[:]
    return bass.AP(tensor=base.tensor, offset=base.offset + off,
                   ap=[list(d) for d in dims])


def _sap(tile_, off, dims):
    """SBUF tile sub-AP: keep partition dim, replace free dims."""
    import concourse.bass as bass
    base = tile_[:]
    return bass.AP(tensor=base.tensor, offset=base.offset + off,
                   ap=[list(base.ap[0])] + [list(d) for d in dims])


def _build_nc():
    import concourse.bass as bass
    import concourse.mybir as mybir
    from concourse.masks import make_identity
    TileContextFixed = _mk_tile_context_fixed()
    f32 = mybir.dt.float32
    f32r = mybir.dt.float32r
    bf16 = mybir.dt.bfloat16
    Copy = mybir.ActivationFunctionType.Copy
    Exp = mybir.ActivationFunctionType.Exp
    ADD = mybir.AluOpType.add
    MUL = mybir.AluOpType.mult
    AX = mybir.AxisListType.X
    THIRD = 1.0 / 3.0

    nc = bass.Bass()
    xs = nc.declare_dram_parameter("xs", [256, H, W], f32, isOutput=False)
    wqkv = nc.declare_dram_parameter("Wqkv", [C, 3 * C], f32, isOutput=False)
    bqkv = nc.declare_dram_parameter("bqkv", [3 * C], f32, isOutput=False)
    wproj = nc.declare_dram_parameter("Wproj", [C, C], f32, isOutput=False)
    bproj = nc.declare_dram_parameter("bproj", [C], f32, isOutput=False)
    out = nc.declare_dram_parameter("out", [C, 64, W], f32, isOutput=True)

    pooled = nc.dram_tensor("pooled", [256 * 64 * 64], f32)
    lout = nc.dram_tensor("lout", [2048 * 512], f32)
    lup = nc.dram_tensor("lup", [256 * H * W], f32)

    with TileContextFixed(nc) as tc:
        with (
            tc.tile_pool(name="consts", bufs=1) as consts,
            tc.tile_pool(name="work", bufs=2) as work,
            tc.tile_pool(name="psum", bufs=2, space="PSUM") as psum,
            tc.tile_pool(name="psumT", bufs=4, space="PSUM") as psumT,
        ):
            # ---- constants ----
            ident = consts.tile([128, 128], f32)
            make_identity(nc, ident[:])
            wqkv_r = consts.tile([128, 4, 1536], f32r)
            wproj_r = consts.tile([128, 4, 512], f32r)
            for kc in range(4):
                wst = work.tile([128, 1536], f32, tag="xwm")
                nc.sync.dma_start(out=wst, in_=_dap(
                    wqkv, kc * 128 * 1536, [[1536, 128], [1, 1536]]))
                nc.vector.tensor_copy(wqkv_r[:, kc, :], wst[:])
                wst2 = work.tile([128, 512], f32, tag="lupg")
                nc.sync.dma_start(out=wst2, in_=_dap(
                    wproj, kc * 128 * 512, [[512, 128], [1, 512]]))
                nc.vector.tensor_copy(wproj_r[:, kc, :], wst2[:])
            bqkv_b = consts.tile([128, 1536], f32)
            nc.sync.dma_start(out=bqkv_b, in_=_dap(bqkv, 0, [[0, 128], [1, 1536]]))
            bproj_b = consts.tile([128, 512], f32)
            nc.sync.dma_start(out=bproj_b, in_=_dap(bproj, 0, [[0, 128], [1, 512]]))

            # ---- stage B: avg-pool 2x2 channel-major -> pooled scratch ----
            # strips of 16 input rows -> 8 pooled rows each
            for cc in range(2):
                for yt in range(8):
                    pin = work.tile([128, 2048], f32, tag="xwm")
                    nc.sync.dma_start(out=pin, in_=_dap(
                        xs, cc * 128 * 16384 + yt * 16 * 128,
                        [[16384, 128], [1, 2048]]))
                    t1 = work.tile([128, 512], f32, tag="lupg")
                    nc.vector.tensor_add(
                        t1[:], _sap(pin, 0, [[256, 8], [2, 64]]),
                        _sap(pin, 1, [[256, 8], [2, 64]]))
                    t2 = work.tile([128, 512], f32, tag="g1")
                    nc.vector.tensor_add(
                        t2[:], _sap(pin, 128, [[256, 8], [2, 64]]),
                        _sap(pin, 129, [[256, 8], [2, 64]]))
                    t3 = work.tile([128, 512], f32, tag="t3")
                    nc.gpsimd.tensor_add(t3[:], t1[:], t2[:])
                    t4 = work.tile([128, 512], f32, tag="outt")
                    nc.scalar.activation(t4[:], t3[:], Copy, scale=0.25)
                    nc.sync.dma_start(
                        out=_dap(pooled, cc * 128 * 4096 + yt * 512,
                                 [[4096, 128], [1, 512]]),
                        in_=t4)

            # ---- shared qkv + attention emitter (window-major tile) ----
            def qkv_attn_tile(x_wm):
                """x_wm: [128 win, 2048] = (di, dj, c). Returns O [128,2048] f32."""
                xt = work.tile([128, 4, 4, 128], f32r, tag="xt")
                for i in range(4):
                    for kc in range(4):
                        psT = psumT.tile([128, 128], f32, tag="psT")
                        nc.tensor.transpose(
                            psT[:], _sap(x_wm, i * 512 + kc * 128, [[1, 128]]),
                            ident[:])
                        nc.scalar.copy(out=xt[:, i, kc, :], in_=psT[:])
                qk = work.tile([128, 4, 1024], bf16, tag="qk")
                vt = work.tile([128, 4, 512], f32, tag="vt")
                for i in range(4):
                    for nb in range(3):
                        psQ = psum.tile([128, 512], f32, tag="psQ")
                        for kc in range(4):
                            nc.tensor.matmul(
                                psQ[:], xt[:, i, kc, :],
                                wqkv_r[:, kc, nb * 512:(nb + 1) * 512],
                                start=(kc == 0), stop=(kc == 3))
                        dst = (qk[:, i, 0:512] if nb == 0 else
                               qk[:, i, 512:1024] if nb == 1 else vt[:, i, :])
                        nc.vector.tensor_add(
                            dst, psQ[:], bqkv_b[:, nb * 512:(nb + 1) * 512])
                # scores S[w,i,n,j] = sum_d q[i,n,d]*k[j,n,d]  (bf16 in, f32 out)
                S = work.tile([128, 128], f32, tag="S")
                tmpb = work.tile([128, 2048], bf16, tag="tmpb")
                for j in range(4):
                    k_j = _sap(qk, j * 1024 + 512, [[0, 4], [64, 8], [1, 64]])
                    q_all = _sap(qk, 0, [[1024, 4], [64, 8], [1, 64]])
                    nc.vector.tensor_mul(tmpb[:], q_all, k_j)
                    nc.vector.tensor_reduce(
                        out=_sap(S, j, [[32, 4], [4, 8]]),
                        in_=_sap(tmpb, 0, [[512, 4], [64, 8], [1, 64]]),
                        axis=AX, op=ADD)
                E = work.tile([128, 128], f32, tag="E")
                nc.scalar.activation(E[:], S[:], Exp, scale=float(HD) ** -0.5)
                D = work.tile([128, 32], f32, tag="D")
                nc.vector.tensor_reduce(
                    out=D[:], in_=_sap(E, 0, [[4, 32], [1, 4]]), axis=AX, op=ADD)
                R = work.tile([128, 32], f32, tag="R")
                nc.vector.reciprocal(R[:], D[:])
                P = work.tile([128, 128], f32, tag="P")
                nc.vector.tensor_mul(P[:], E[:], _sap(R, 0, [[1, 32], [0, 4]]))
                # O[w,i,n,d] = sum_j P[w,i,n,j] * v[w,j,n,d]
                O = work.tile([128, 2048], f32, tag="O")
                tmpO = work.tile([128, 2048], f32, tag="tmpO")
                for j in range(4):
                    p_j = _sap(P, j, [[32, 4], [4, 8], [0, 64]])
                    v_j = _sap(vt, j * 512, [[0, 4], [64, 8], [1, 64]])
                    if j == 0:
                        nc.vector.tensor_mul(O[:], p_j, v_j)
                    else:
                        nc.vector.tensor_mul(tmpO[:], p_j, v_j)
                        nc.gpsimd.tensor_add(O[:], O[:], tmpO[:])
                return O

            # ---- stage C: local attention (4 ltiles) -> lout scratch ----
            for lt in range(4):
                x_l = work.tile([128, 2, 1024], f32, tag="xwm")
                for di in range(2):
                    nc.sync.dma_start(out=x_l[:, di, :], in_=_dap(
                        pooled, lt * 262144 + di * 32768,
                        [[65536, 4], [1024, 32], [1, 1024]]))
                O_l = qkv_attn_tile(x_l)
                for di in range(2):
                    nc.sync.dma_start(
                        out=_dap(lout, lt * 262144 + di * 32768,
                                 [[65536, 4], [1024, 32], [1, 1024]]),
                        in_=_sap(O_l, di * 1024, [[1, 1024]]))

            # ---- stage D: smoothing + bilinear x2 upsample -> lup scratch ----
            # All scale factors folded algebraically: with raw sums
            #   a1[y] = l[y-1]+l[y] (reflect y=0), a2[y] = l[y]+l[y+1] (reflect
            #   x=63), sraw = a1+a2 (= 2*(lx+ly))
            #   u'[2y] = sraw[y] + sraw[y-1]/3, u'[2y+1] = sraw[y] + sraw[y+1]/3
            #   L'[2x] = u'[x] + u'[x-1]/3,  L'[2x+1] = u'[x] + u'[x+1]/3
            #   lup = 0.28125 * L'   (= 0.5 * 0.375 * 0.75 * ... collapsed)
            # borders use clamped taps; the stt form works there unchanged.
            for cc in range(2):
                for st in range(4):          # strips of 16 pooled rows
                    y0 = st * 16
                    r0, r1 = max(y0 - 2, 0), min(y0 + 17, 64)   # Lp rows
                    s0, s1 = max(y0 - 1, 0), min(y0 + 17, 64)   # sraw rows
                    nlr = r1 - r0
                    nsr = s1 - s0
                    Lp = work.tile([128, nlr * 64], f32, tag="xwm")
                    nc.sync.dma_start(out=Lp, in_=_dap(
                        lout, cc * 128 * 4096 + r0 * 64,
                        [[4096, 128], [1, nlr * 64]]))

                    def lrow(y):  # strip-local Lp row offset
                        return (y - r0) * 64

                    def srow(y):  # strip-local sraw row offset
                        return (y - s0) * 64

                    a1 = work.tile([128, nsr * 64], f32, tag="tmpO")
                    ym = max(s0, 1)  # main region rows [ym, s1)
                    nc.vector.tensor_add(
                        _sap(a1, srow(ym), [[1, (s1 - ym) * 64]]),
                        _sap(Lp, lrow(ym - 1), [[1, (s1 - ym) * 64]]),
                        _sap(Lp, lrow(ym), [[1, (s1 - ym) * 64]]))
                    if s0 == 0:  # reflect top: a1[0] = l[0] + l[1]
                        nc.vector.tensor_add(
                            _sap(a1, 0, [[1, 64]]),
                            _sap(Lp, 0, [[1, 64]]),
                            _sap(Lp, 64, [[1, 64]]))
                    a2 = work.tile([128, nsr * 64], f32, tag="O")
                    nc.gpsimd.tensor_add(
                        _sap(a2, 0, [[64, nsr], [1, 63]]),
                        _sap(Lp, lrow(s0), [[64, nsr], [1, 63]]),
                        _sap(Lp, lrow(s0) + 1, [[64, nsr], [1, 63]]))
                    nc.gpsimd.tensor_add(
                        _sap(a2, 63, [[64, nsr]]),
                        _sap(Lp, lrow(s0) + 63, [[64, nsr]]),
                        _sap(Lp, lrow(s0) + 62, [[64, nsr]]))
                    sraw = work.tile([128, nsr * 64], f32, tag="xwm")
                    nc.vector.tensor_add(sraw[:], a1[:], a2[:])
                    # y-upsample (u' rows Y-2*y0, 32 rows x 64 cols)
                    u = work.tile([128, 2048], f32, tag="vt")
                    ye = max(y0, 1)  # even rows needing y-1
                    nc.vector.scalar_tensor_tensor(
                        out=_sap(u, (ye - y0) * 128, [[128, y0 + 16 - ye], [1, 64]]),
                        in0=_sap(sraw, srow(ye - 1), [[64, y0 + 16 - ye], [1, 64]]),
                        scalar=THIRD,
                        in1=_sap(sraw, srow(ye), [[64, y0 + 16 - ye], [1, 64]]),
                        op0=MUL, op1=ADD)
                    if y0 == 0:  # Y=0: taps both row 0
                        nc.vector.scalar_tensor_tensor(
                            out=_sap(u, 0, [[1, 64]]),
                            in0=_sap(sraw, 0, [[1, 64]]), scalar=THIRD,
                            in1=_sap(sraw, 0, [[1, 64]]), op0=MUL, op1=ADD)
                    yo1 = min(y0 + 16, 63)  # odd rows needing y+1: y in [y0, yo1)
                    nc.vector.scalar_tensor_tensor(
                        out=_sap(u, 64, [[128, yo1 - y0], [1, 64]]),
                        in0=_sap(sraw, srow(y0 + 1), [[64, yo1 - y0], [1, 64]]),
                        scalar=THIRD,
                        in1=_sap(sraw, srow(y0), [[64, yo1 - y0], [1, 64]]),
                        op0=MUL, op1=ADD)
                    if y0 + 16 == 64:  # Y=127: taps both row 63
                        nc.vector.scalar_tensor_tensor(
                            out=_sap(u, 31 * 64, [[1, 64]]),
                            in0=_sap(sraw, srow(63), [[1, 64]]), scalar=THIRD,
                            in1=_sap(sraw, srow(63), [[1, 64]]), op0=MUL, op1=ADD)
                    # x-upsample per 16-row half + final scale + store
                    for hf in range(2):
                        Lh = work.tile([128, 2048], f32, tag="O")
                        ub = hf * 16 * 64  # u offset of this half's rows
                        nc.vector.scalar_tensor_tensor(
                            out=_sap(Lh, 2, [[128, 16], [2, 63]]),
                            in0=_sap(u, ub, [[64, 16], [1, 63]]), scalar=THIRD,
                            in1=_sap(u, ub + 1, [[64, 16], [1, 63]]),
                            op0=MUL, op1=ADD)
                        nc.vector.scalar_tensor_tensor(
                            out=_sap(Lh, 0, [[128, 16]]),
                            in0=_sap(u, ub, [[64, 16]]), scalar=THIRD,
                            in1=_sap(u, ub, [[64, 16]]), op0=MUL, op1=ADD)
                        nc.vector.scalar_tensor_tensor(
                            out=_sap(Lh, 1, [[128, 16], [2, 63]]),
                            in0=_sap(u, ub + 1, [[64, 16], [1, 63]]), scalar=THIRD,
                            in1=_sap(u, ub, [[64, 16], [1, 63]]),
                            op0=MUL, op1=ADD)
                        nc.vector.scalar_tensor_tensor(
                            out=_sap(Lh, 127, [[128, 16]]),
                            in0=_sap(u, ub + 63, [[64, 16]]), scalar=THIRD,
                            in1=_sap(u, ub + 63, [[64, 16]]), op0=MUL, op1=ADD)
                        Lsc = work.tile([128, 2048], f32, tag="tmpO")
                        nc.scalar.activation(Lsc[:], Lh[:], Copy, scale=0.28125)
                        nc.sync.dma_start(
                            out=_dap(lup,
                                     cc * 128 * 16384 + (2 * y0 + 16 * hf) * 128,
                                     [[16384, 128], [1, 2048]]),
                            in_=Lsc)

            # ---- stage A: global path (16 wtiles) ----
            for ti in range(16):
                x_wm = work.tile([128, 2, 1024], f32, tag="xwmA")
                for di in range(2):
                    nc.sync.dma_start(out=x_wm[:, di, :], in_=_dap(
                        xs, ti * 4 * 65536 + di * 65536,
                        [[131072, 2], [1024, 64], [1, 1024]]))
                O = qkv_attn_tile(x_wm)
                sums = work.tile([128, 4, 512], f32, tag="sums")
                for i in range(4):
                    at = work.tile([128, 4, 128], f32r, tag="at")
                    for kc in range(4):
                        psT2 = psumT.tile([128, 128], f32, tag="psT")
                        nc.tensor.transpose(
                            psT2[:], _sap(O, i * 512 + kc * 128, [[1, 128]]),
                            ident[:])
                        nc.scalar.copy(out=at[:, kc, :], in_=psT2[:])
                    psP = psum.tile([128, 512], f32, tag="psQ")
                    for kc in range(4):
                        nc.tensor.matmul(psP[:], at[:, kc, :], wproj_r[:, kc, :],
                                         start=(kc == 0), stop=(kc == 3))
                    di, dj = i >> 1, i & 1
                    lupg = work.tile([128, 512], f32, tag="lupg")
                    nc.sync.dma_start(out=lupg, in_=_dap(
                        lup, ti * 262144 + di * 65536 + dj * 512,
                        [[131072, 2], [1024, 64], [1, 512]]))
                    g1 = work.tile([128, 512], f32, tag="g1")
                    nc.vector.tensor_add(g1[:], psP[:], bproj_b[:])
                    nc.gpsimd.tensor_add(sums[:, i, :], g1[:], lupg[:])
                # final transpose to (C, h, w) + DMA out
                for ch in range(4):
                    outt = work.tile([128, 512], f32, tag="outt")
                    for i in range(4):
                        di, dj = i >> 1, i & 1
                        psF = psumT.tile([128, 128], f32, tag="psT")
                        nc.tensor.transpose(
                            psF[:], _sap(sums, i * 512 + ch * 128, [[1, 128]]),
                            ident[:])
                        nc.scalar.copy(
                            out=_sap(outt, di * 128 + dj, [[256, 2], [2, 64]]),
                            in_=_sap(psF, 0, [[64, 2], [1, 64]]))
                    nc.sync.dma_start(
                        out=_dap(out, ch * 128 * 8192 + ti * 512,
                                 [[8192, 128], [128, 4], [1, 128]]),
                        in_=outt)
    return nc


def _get_nc():
    if 'nc' not in _RUN_CACHE:
        _RUN_CACHE['nc'] = _build_nc()
    return _RUN_CACHE['nc']


def kernel(**inputs):
    from concourse.bass_utils import run_bass_kernel_spmd
    x = np.ascontiguousarray(np.asarray(inputs['x'], dtype=np.float32))
    Wqkv = np.ascontiguousarray(np.asarray(inputs['Wqkv'], dtype=np.float32))
    bqkv = np.ascontiguousarray(np.asarray(inputs['bqkv'], dtype=np.float32))
    Wproj = np.ascontiguousarray(np.asarray(inputs['Wproj'], dtype=np.float32))
    bproj = np.ascontiguousarray(np.asarray(inputs['bproj'], dtype=np.float32))

    nc = _get_nc()
    in_maps = []
    shards = []
    for b in range(B):
        for half in range(2):
            shards.append((b, half))
            in_maps.append({
                "xs": np.ascontiguousarray(x[b, 256 * half:256 * (half + 1)]),
                "Wqkv": Wqkv, "bqkv": bqkv, "Wproj": Wproj, "bproj": bproj,
            })
    r = run_bass_kernel_spmd(nc, in_maps, core_ids=list(range(8)))
    _RUN_CACHE['last_result'] = r
    full = np.empty((B, C, H, W), dtype=np.float32)
    for (b, half), res in zip(shards, r.results):
        full[b, :, 64 * half:64 * (half + 1), :] = res["out"]
    return full
